# revision 28
# baseline (speedup 1.0000x reference)
"""CQAttention (QANet context-query attention) Trainium2 kernel, v2d.

Problem: B=64, H=256, Lc=2048, Lq=256.
  S[b,i,j] = r[i] + c[j] + S3[i,j],  S3 = sum_h Ct[i,h]*w3[h]*Qt[j,h]
  S_row = softmax_j(masked by qmask), S_col = softmax_i(masked by cmask)
  A = S_row @ Qt ; Bt = S_row @ (S_col^T @ Ct)
  out[b] = [Ct; A; Ct*A; Ct*Bt]^T  -> [B, 4H, Lc]

Strategy (data-parallel, 8 batches/core, all fp16 matmuls):
  - P^T[j,i] = exp(S3^T + cb_j) with cb = c + ln(vq) - ln(32) folded into the
    ACT bias (row-softmax weights live inside P^T; they cancel in the column
    softmax, so one exp serves both paths).
  - e^{r_i}*vc/4 folded into the host-prescaled Ct operand (cancels in the
    column-softmax ratio), giving X = S_col^T @ Ct directly.
  - P[i,j] via PE transposes of P^T. A_num^T = Qt @ P^T, Bt_num^T = X @ P^T,
    rowsum = 1 @ P^T; unnormalized numerators shipped fp16 + rowsum f32;
    host divides and assembles [C; A; C*A; C*Bt] (C section is the input).
  - Cross-batch software pipelining keeps the PE fed while ACT/DVE evict.
"""

import numpy as np

B, H, LC, LQ = 64, 256, 2048, 256
NCORES = 8
NB = B // NCORES

KC = 2    # h chunks of 128
JC = 2    # j chunks of 128
IC = 16   # i chunks of 128
IT = 4    # i tiles of 512
HA = H + 1
LAM = np.log(32.0)
S_ER = 4.0
NEG = 1.0e30

_CACHE = {}


def _build():
    import concourse.bacc as bacc
    import concourse.mybir as mybir
    import concourse.tile as tile
    from contextlib import ExitStack

    F32 = mybir.dt.float32
    F16 = mybir.dt.float16
    AF = mybir.ActivationFunctionType
    MUL = mybir.AluOpType.mult
    ADD = mybir.AluOpType.add

    nc = bacc.Bacc("TRN2", target_bir_lowering=False, debug=False,
                   enable_asserts=False)

    c16 = nc.dram_tensor("c16", [NB, 128, KC, LC], F16, kind="ExternalInput").ap()
    q316 = nc.dram_tensor("q316", [NB, 128, KC, LQ], F16, kind="ExternalInput").ap()
    qt16 = nc.dram_tensor("qt16", [NB, 128, JC, H], F16, kind="ExternalInput").ap()
    cb = nc.dram_tensor("cb", [NB, 128, JC], F32, kind="ExternalInput").ap()
    omv = nc.dram_tensor("omv", [NB, 128, JC], F32, kind="ExternalInput").ap()
    ct16 = nc.dram_tensor("ct16", [NB, 128, IC, HA], F16, kind="ExternalInput").ap()
    kid = nc.dram_tensor("kid", [128, 128], F16, kind="ExternalInput").ap()
    a16 = nc.dram_tensor("a16", [NB, 128, KC, LC], F16, kind="ExternalOutput").ap()
    b16 = nc.dram_tensor("b16", [NB, 128, KC, LC], F16, kind="ExternalOutput").ap()
    rs = nc.dram_tensor("rs", [NB, 128, IC], F32, kind="ExternalOutput").ap()

    with tile.TileContext(nc) as tc:
        with ExitStack() as ctx:
            konst = ctx.enter_context(tc.tile_pool(name="konst", bufs=1))
            cpool = ctx.enter_context(tc.tile_pool(name="cpool", bufs=2))
            qpool = ctx.enter_context(tc.tile_pool(name="qpool", bufs=2))
            ppool = ctx.enter_context(tc.tile_pool(name="ppool", bufs=2))
            ptpool = ctx.enter_context(tc.tile_pool(name="ptpool", bufs=2))
            xpool = ctx.enter_context(tc.tile_pool(name="xpool", bufs=2))
            opool = ctx.enter_context(tc.tile_pool(name="opool", bufs=2))
            small = ctx.enter_context(tc.tile_pool(name="small", bufs=4))
            s3_ps = ctx.enter_context(tc.tile_pool(name="s3_ps", bufs=2, space="PSUM"))
            tp_ps = ctx.enter_context(tc.tile_pool(name="tp_ps", bufs=1, space="PSUM"))
            x_ps = ctx.enter_context(tc.tile_pool(name="x_ps", bufs=1, space="PSUM"))
            mm_ps = ctx.enter_context(tc.tile_pool(name="mm_ps", bufs=4, space="PSUM"))

            kid_sb = konst.tile([128, 128], F16)
            nc.sync.dma_start(kid_sb[:], kid[:])

            def load_batch(b):
                csb = cpool.tile([128, KC * LC], F16, tag="csb")
                nc.sync.dma_start(
                    csb[:].rearrange("p (c i) -> p c i", c=KC), c16[b])
                q3sb = qpool.tile([128, KC * LQ], F16, tag="q3sb")
                nc.sync.dma_start(
                    q3sb[:].rearrange("p (c j) -> p c j", c=KC), q316[b])
                qtsb = qpool.tile([128, JC * H], F16, tag="qtsb")
                nc.sync.dma_start(
                    qtsb[:].rearrange("p (c h) -> p c h", c=JC), qt16[b])
                cbsb = qpool.tile([128, JC], F32, tag="cbsb")
                nc.sync.dma_start(cbsb[:], cb[b])
                omsb = qpool.tile([128, JC], F32, tag="omsb")
                nc.sync.dma_start(omsb[:], omv[b])
                ctsb = cpool.tile([128, IC * HA], F16, tag="ctsb")
                nc.sync.dma_start(
                    ctsb[:].rearrange("p (c h) -> p c h", c=IC), ct16[b])
                return csb, q3sb, qtsb, cbsb, omsb, ctsb

            def stage1(b, tiles):
                """S3^T on PE -> exp(+cb bias) -> P^T; transposes -> P."""
                csb, q3sb, qtsb, cbsb, omsb, ctsb = tiles
                c3 = csb[:].rearrange("p (c i) -> p c i", c=KC)
                q33 = q3sb[:].rearrange("p (c j) -> p c j", c=KC)

                pt = ptpool.tile([128, JC * LC], F16, tag="pt")
                pt3 = pt[:].rearrange("p (c i) -> p c i", c=JC)
                for jc in range(JC):
                    for it in range(IT):
                        ps = s3_ps.tile([128, 512], F32, tag="s3")
                        for kc in range(KC):
                            nc.tensor.matmul(
                                ps[:],
                                q33[:, kc, jc * 128:(jc + 1) * 128],
                                c3[:, kc, it * 512:(it + 1) * 512],
                                start=(kc == 0), stop=(kc == KC - 1))
                        nc.scalar.activation(
                            pt3[:, jc, it * 512:(it + 1) * 512], ps[:], AF.Exp,
                            bias=cbsb[:, jc:jc + 1])

                # P [i,j] via PE transposes of P^T; rowsum rides the ACT
                # eviction as accum_out (free-axis sum over all j per ic).
                p16 = ppool.tile([128, IC * LQ], F16, tag="p16")
                p3 = p16[:].rearrange("p (c j) -> p c j", c=IC)
                rsT = small.tile([128, IC], F32, tag="rsT")
                for g in range(IC // 2):
                    tp = tp_ps.tile([128, 512], F16, tag="tp")
                    for u in range(4):
                        ic, jc = 2 * g + u // 2, u % 2
                        nc.tensor.transpose(
                            tp[:, u * 128:(u + 1) * 128],
                            pt3[:, jc, ic * 128:(ic + 1) * 128],
                            kid_sb[:])
                    for v in range(2):
                        ic = 2 * g + v
                        nc.scalar.activation(
                            p3[:, ic, :], tp[:, v * 256:(v + 1) * 256],
                            AF.Copy, accum_out=rsT[:, ic:ic + 1])
                nc.sync.dma_start(rs[b], rsT[:])
                return pt, p16

            def stage2(b, tiles, mats):
                """Row sums, X, A, Bt, stores."""
                csb, q3sb, qtsb, cbsb, omsb, ctsb = tiles
                pt, p16 = mats
                qt3 = qtsb[:].rearrange("p (c h) -> p c h", c=JC)
                ct3 = ctsb[:].rearrange("p (c h) -> p c h", c=IC)
                pt3 = pt[:].rearrange("p (c i) -> p c i", c=JC)
                p3 = p16[:].rearrange("p (c j) -> p c j", c=IC)

                # X = S_col^T @ Ct via the er-folded augmented Ct
                x16 = xpool.tile([128, JC * H], F16, tag="x16")
                x3 = x16[:].rearrange("p (c h) -> p c h", c=JC)
                for jc in range(JC):
                    xps = x_ps.tile([128, HA], F32, tag="x")
                    for ic in range(IC):
                        nc.tensor.matmul(
                            xps[:],
                            p3[:, ic, jc * 128:(jc + 1) * 128],
                            ct3[:, ic, :],
                            start=(ic == 0), stop=(ic == IC - 1))
                    cs = small.tile([128, 1], F32, tag="cs")
                    nc.vector.tensor_tensor(
                        cs[:], xps[:, H:HA], omsb[:, jc:jc + 1], ADD)
                    colr = small.tile([128, 1], F32, tag="colr")
                    nc.vector.reciprocal_approx_fast(colr[:], cs[:])
                    nc.vector.tensor_scalar_mul(
                        x3[:, jc, :], xps[:, 0:H], colr[:])

                # A^T / Bt^T numerators; plain-copy eviction (ACT / DVE)
                asb = opool.tile([128, KC * LC], F16, tag="asb")
                bsb = opool.tile([128, KC * LC], F16, tag="bsb")
                a3 = asb[:].rearrange("p (c i) -> p c i", c=KC)
                b3 = bsb[:].rearrange("p (c i) -> p c i", c=KC)
                for hc in range(KC):
                    for it in range(IT):
                        i0, i1 = it * 512, (it + 1) * 512
                        aps = mm_ps.tile([128, 512], F32, tag="mm")
                        for jc in range(JC):
                            nc.tensor.matmul(
                                aps[:],
                                qt3[:, jc, hc * 128:(hc + 1) * 128],
                                pt3[:, jc, i0:i1],
                                start=(jc == 0), stop=(jc == JC - 1))
                        bps = mm_ps.tile([128, 512], F32, tag="mm")
                        for jc in range(JC):
                            nc.tensor.matmul(
                                bps[:],
                                x3[:, jc, hc * 128:(hc + 1) * 128],
                                pt3[:, jc, i0:i1],
                                start=(jc == 0), stop=(jc == JC - 1))
                        nc.vector.tensor_copy(a3[:, hc, i0:i1], aps[:])
                        nc.vector.tensor_copy(b3[:, hc, i0:i1], bps[:])

                nc.sync.dma_start(a16[b], a3[:, :, :])
                nc.sync.dma_start(b16[b], b3[:, :, :])

            # software pipeline: stage1(b+1) on PE while b's downstream waits
            tiles = load_batch(0)
            mats = stage1(0, tiles)
            for b in range(NB):
                prev = (tiles, mats)
                if b + 1 < NB:
                    tiles = load_batch(b + 1)
                    mats = stage1(b + 1, tiles)
                stage2(b, *prev)

    nc.compile()
    return nc


def _prep(C, Q, cmask, qmask, line_project):
    w1, w2, w3 = np.split(line_project.astype(np.float64), 3)
    r = np.einsum('bhi,h->bi', C.astype(np.float64), w1).astype(np.float32)
    c_ = np.einsum('bhj,h->bj', Q.astype(np.float64), w2).astype(np.float32)
    vq = 1.0 - qmask
    vc = 1.0 - cmask
    # cb = c + ln(vq) - ln(32): -inf (=-1e30) on masked j
    cbv = np.where(vq > 0.5, c_ - LAM, -NEG).astype(np.float32)
    er = (np.exp(r) * vc / S_ER).astype(np.float32)           # [B, LC]

    # [B, 128, KC, LC]: h = kc*128 + p
    c16 = np.ascontiguousarray(
        C.reshape(B, KC, 128, LC).transpose(0, 2, 1, 3)).astype(np.float16)
    q3 = (Q * w3.astype(np.float32)[None, :, None])
    q316 = np.ascontiguousarray(
        q3.reshape(B, KC, 128, LQ).transpose(0, 2, 1, 3)).astype(np.float16)
    # [B, 128, JC, H]: j = jc*128 + p
    qt16 = np.ascontiguousarray(
        Q.transpose(0, 2, 1).reshape(B, JC, 128, H).transpose(0, 2, 1, 3)
    ).astype(np.float16)
    cbt = np.ascontiguousarray(
        cbv.reshape(B, JC, 128).transpose(0, 2, 1)).astype(np.float32)
    omvt = np.ascontiguousarray(
        qmask.reshape(B, JC, 128).transpose(0, 2, 1)).astype(np.float32)
    # [B, 128, IC, HA]: i = ic*128 + p
    cta = np.empty((B, LC, HA), np.float32)
    cta[:, :, 0:H] = C.transpose(0, 2, 1) * er[:, :, None]
    cta[:, :, H] = er
    ct16 = np.ascontiguousarray(
        cta.reshape(B, IC, 128, HA).transpose(0, 2, 1, 3)).astype(np.float16)
    return c16, q316, qt16, cbt, omvt, ct16


def make_in_maps(C, Q, cmask, qmask, line_project):
    C = np.asarray(C, dtype=np.float32)
    Q = np.asarray(Q, dtype=np.float32)
    cmask = np.asarray(cmask, dtype=np.float32)
    qmask = np.asarray(qmask, dtype=np.float32)
    line_project = np.asarray(line_project, dtype=np.float32)
    c16, q316, qt16, cbt, omvt, ct16 = _prep(C, Q, cmask, qmask, line_project)
    kid = np.eye(128, dtype=np.float16)
    in_maps = []
    for core in range(NCORES):
        s = slice(core * NB, (core + 1) * NB)
        in_maps.append({
            "c16": c16[s], "q316": q316[s], "qt16": qt16[s],
            "cb": cbt[s], "omv": omvt[s], "ct16": ct16[s], "kid": kid,
        })
    return in_maps


def kernel(C, Q, cmask, qmask, line_project):
    from concourse.bass_utils import run_bass_kernel_spmd

    C = np.asarray(C, dtype=np.float32)
    in_maps = make_in_maps(C, Q, cmask, qmask, line_project)
    if "nc" not in _CACHE:
        _CACHE["nc"] = _build()
    nc = _CACHE["nc"]
    res = run_bass_kernel_spmd(nc, in_maps, core_ids=list(range(NCORES)))
    a16 = np.concatenate([res.results[c]["a16"] for c in range(NCORES)], axis=0)
    b16 = np.concatenate([res.results[c]["b16"] for c in range(NCORES)], axis=0)
    rsv = np.concatenate([res.results[c]["rs"] for c in range(NCORES)], axis=0)
    # rs [B, 128, IC] (i = ic*128+p) -> [B, LC]
    rsv = rsv.transpose(0, 2, 1).reshape(B, LC)
    # [B, 128, KC, LC] (h = kc*128+p) -> [B, H, LC]; normalize by rowsums
    rr = (1.0 / rsv)[:, None, :]
    A = a16.transpose(0, 2, 1, 3).reshape(B, H, LC).astype(np.float32) * rr
    Bt = b16.transpose(0, 2, 1, 3).reshape(B, H, LC).astype(np.float32) * rr
    out = np.empty((B, 4 * H, LC), np.float32)
    out[:, 0:H] = C
    out[:, H:2 * H] = A
    out[:, 2 * H:3 * H] = C * A
    out[:, 3 * H:4 * H] = C * Bt
    return out
